# revision 7
# baseline (speedup 1.0000x reference)
"""Multi-head attention Trainium2 kernel (8 NeuronCores, data-parallel over batch).

Per-core program (2 batches per core), optimized for the TimelineSim cost
model (matmul charged = out_free_size x cycles_per_row; M and K are free;
fp8 DoubleRow = 0.5 cycles/row over 2 K-chunks; every instruction carries
tens-of-ns sequencer/semaphore overhead).

Structure:
  x arrives split hi/lo fp8e4m3 (x = xh + xl exactly at bf16-grade precision)
  -> QKV projections via fp8 DoubleRow in 3 passes (xh*Wh + xh*Wl + xl*Wh):
     1536 rows per 512-wide chunk group vs 2048 bf16 (-25% PE).  Q/K/V
     weights are host-scaled by 16 so the fp8 split clears e4m3 subnormals;
     the 256x on scores folds into the exp scale, the 16x on V folds into
     Wproj (host-divided).
  -> scores S^T [s, t] per (head, s-tile): ONE bf16 matmul [128, 1024]
  -> exp split across three engines (ACT exact; DVE/Pool via bf16
     Schraudolph: one fused tensor_scalar mult+add -> int16, bitcast bf16;
     ~10/128 tiles each) so the ACT stream ends before PE and the tail
     projection never stalls the span.
  -> PV in O-form with fused rowsum column (bf16), reversed-j accumulation
  -> normalize via DVE reciprocal + broadcast multiply -> osb bf16
  -> O^T via bf16 PE transposes -> output projection (Wproj/16) + bias
PE is the binding engine (~131us); ACT ~112us, DVE ~46us, Pool ~48us.
"""
import sys
import os

sys.path.insert(0, "/opt/trn_rl_repo")
import numpy as np
import ml_dtypes

B, C, HH, WW = 16, 512, 32, 32
T = HH * WW              # 1024
NH, HD = 8, 64
BL = 2                   # batches per core
NCORES = 8

WSCALE = 16.0            # host scale on Wq/Wk/Wv for fp8 hi/lo split
EXP_SCALE = 0.125 / (WSCALE * WSCALE)          # folds q,k scaling into exp
SCHRAU_A = EXP_SCALE * 128.0 / float(np.log(2.0))
SCHRAU_B = 16256.0 - 7.0                       # bf16 one bias + centering adj

N_S_PAIRS = 6            # of 64 j-pair units, computed via DVE Schraudolph

_CACHE = {}


def _bf16(a):
    """f32 -> bf16 bits (round to nearest even), as uint16."""
    u = np.ascontiguousarray(a, dtype=np.float32).view(np.uint32)
    r = (u + 0x7FFF + ((u >> 16) & 1)) >> 16
    return r.astype(np.uint16)


def _e4_split(a):
    """f32 -> (hi, lo) float8_e4m3 byte arrays with a == hi + lo approx."""
    hi = a.astype(ml_dtypes.float8_e4m3)
    lo = (a - hi.astype(np.float32)).astype(ml_dtypes.float8_e4m3)
    return hi.view(np.uint8), lo.view(np.uint8)


def _s_pair_engines():
    """Map pair unit u (0..63) -> None (ACT) or alternating 'dve'/'pool'."""
    out = {}
    s_count = 0
    for u in range(64):
        if (u * N_S_PAIRS) // 64 != ((u + 1) * N_S_PAIRS) // 64:
            out[u] = ("dve", "dve")
            s_count += 1
        else:
            out[u] = None
    return out


def _build_nc():
    import concourse.bacc as bacc
    import concourse.mybir as mybir
    import concourse.tile as tile
    from concourse import masks

    f32 = mybir.dt.float32
    bf16 = mybir.dt.bfloat16
    u16 = mybir.dt.uint16
    u8 = mybir.dt.uint8
    i16 = mybir.dt.int16
    e4 = mybir.dt.float8e4
    Exp = mybir.ActivationFunctionType.Exp
    DR = mybir.MatmulPerfMode.DoubleRow
    Mult = mybir.AluOpType.mult
    Add = mybir.AluOpType.add

    nc = bacc.Bacc("TRN2", target_bir_lowering=False, debug=False, num_devices=NCORES)
    xh_d = nc.dram_tensor("xh", [C, BL * T], u8, kind="ExternalInput").ap()
    xl_d = nc.dram_tensor("xl", [C, BL * T], u8, kind="ExternalInput").ap()
    w_d = {}
    for nm in ("wqh", "wql", "wkh", "wkl", "wvh", "wvl"):
        w_d[nm] = nc.dram_tensor(nm, [128, 2048], u8, kind="ExternalInput").ap()
    wp = nc.dram_tensor("wp", [128, 2048], u16, kind="ExternalInput").ap()
    bp = nc.dram_tensor("bp", [1, C], f32, kind="ExternalInput").ap()
    y = nc.dram_tensor("y", [BL * T, C], u16, kind="ExternalOutput").ap()

    s_eng = _s_pair_engines()

    with tile.TileContext(nc) as tc:
        with tc.tile_pool(name="const", bufs=1) as cpool, \
             tc.tile_pool(name="qk", bufs=2) as qk_pool, \
             tc.tile_pool(name="vv", bufs=2) as v_pool, \
             tc.tile_pool(name="pp", bufs=4) as p_pool, \
             tc.tile_pool(name="ob", bufs=2) as o_pool, \
             tc.tile_pool(name="ot", bufs=2) as ot_pool, \
             tc.tile_pool(name="yy", bufs=8) as y_pool, \
             tc.tile_pool(name="rr", bufs=3) as r_pool, \
             tc.tile_pool(name="psA", bufs=3, space="PSUM") as psA, \
             tc.tile_pool(name="psB", bufs=2, space="PSUM") as psB:

            # ---- constants + weights ----
            xts = {}
            for pname in ("h", "l"):
                xts[pname] = cpool.tile([128, 4, BL * T], e4, tag=f"xt{pname}",
                                        name=f"xts_{pname}")
            xsrc = {"h": xh_d.bitcast(e4).rearrange("(cc p) t -> p cc t", cc=4),
                    "l": xl_d.bitcast(e4).rearrange("(cc p) t -> p cc t", cc=4)}
            ws = {nm: cpool.tile([128, 2048], e4, tag=nm, name=f"ws_{nm}")
                  for nm in ("wqh", "wql", "wkh", "wkl", "wvh", "wvl")}
            wp_s = cpool.tile([128, 2048], bf16, tag="wp")
            bias_b = cpool.tile([128, C], f32, tag="bias")
            ident = cpool.tile([128, 128], bf16, tag="ident")
            masks.make_identity(nc, ident[:])

            # ACT: trigger the Exp table load immediately (before any exp
            # work is possible) with a 1-element dummy activation.
            dummy = cpool.tile([128, 1], f32, tag="dummy")
            nc.scalar.activation(dummy[:], ident[:, 0:1], Exp, scale=1.0)

            # Prologue DMAs. Gating chain for the first scores: wq{h,l}/wk{h,l}
            # pair-0 cols + x{h,l} t 0:1024 -- big chunks on SP (fewest 565ns
            # issues), k-weights on ACT, all bulk transfers via Pool SWDGE
            # (idle engine, ~1.1us each, not latency-critical).
            nc.sync.dma_start(ws["wqh"][:, 0:512], w_d["wqh"].bitcast(e4)[:, 0:512])
            nc.scalar.dma_start(ws["wkh"][:, 0:512], w_d["wkh"].bitcast(e4)[:, 0:512])
            nc.sync.dma_start(xts["h"][:, :, 0:1024], xsrc["h"][:, :, 0:1024])
            nc.scalar.dma_start(ws["wkl"][:, 0:512], w_d["wkl"].bitcast(e4)[:, 0:512])
            nc.sync.dma_start(xts["l"][:, :, 0:1024], xsrc["l"][:, :, 0:1024])
            nc.sync.dma_start(ws["wql"][:, 0:512], w_d["wql"].bitcast(e4)[:, 0:512])
            nc.sync.dma_start(ws["wvh"][:], w_d["wvh"].bitcast(e4))
            nc.sync.dma_start(ws["wvl"][:], w_d["wvl"].bitcast(e4))
            for nm in ("wqh", "wkh", "wql", "wkl"):
                nc.gpsimd.dma_start(ws[nm][:, 512:2048], w_d[nm].bitcast(e4)[:, 512:2048])
            for pname in ("h", "l"):
                nc.gpsimd.dma_start(xts[pname][:, :, 1024:2048],
                                    xsrc[pname][:, :, 1024:2048])
            nc.gpsimd.dma_start(wp_s[:], wp.bitcast(bf16))
            nc.gpsimd.dma_start(bias_b[:], bp.to_broadcast([128, C]))

            # ---- PE warmup: ramp the pstate clock during the DMA prologue
            wps = psB.tile([128, 128], bf16, tag="B", name="warm")
            for i in range(14):
                nc.tensor.transpose(wps[:], ident[:], ident[:])

            def dr3(ps, stat_hi, stat_lo, mov_hi, mov_lo):
                """3-pass hi/lo fp8 DoubleRow accumulation group.
                stat/mov are fns: cc2 -> AP ([128, 2, m] / [128, 2, n])."""
                passes = [(stat_hi, mov_hi), (stat_hi, mov_lo), (stat_lo, mov_hi)]
                n = 0
                for sf, mf in passes:
                    for cc2 in range(2):
                        nc.tensor.matmul(ps, sf(cc2), mf(cc2),
                                         start=(n == 0), stop=(n == 5),
                                         perf_mode=DR)
                        n += 1

            def qkv_qk_chunk(b, p, wi, ch, qts, kts, cengine=None):
                # one 512-wide t-chunk of Q^T or K^T for head-pair p.
                whi, wlo = (ws["wqh"], ws["wql"]) if wi == 0 else (ws["wkh"], ws["wkl"])
                dst = qts if wi == 0 else kts
                ps = psB.tile([128, 512], f32, tag="B", name=f"qk_{b}_{p}_{wi}_{ch}")
                t0 = b * T + ch * 512

                def stat(w):
                    return lambda cc2: w[:, p * 512 + cc2 * 256: p * 512 + cc2 * 256 + 256] \
                        .rearrange("p (two m) -> p two m", two=2)

                def mov(xp):
                    return lambda cc2: xts[xp][:, 2 * cc2:2 * cc2 + 2, t0:t0 + 512]

                dr3(ps[:], stat(whi), stat(wlo), mov("h"), mov("l"))
                if cengine == "act":
                    nc.scalar.copy(dst[:, p, ch * 512:(ch + 1) * 512], ps[:])
                else:
                    nc.vector.tensor_copy(dst[:, p, ch * 512:(ch + 1) * 512], ps[:])

            def qkv_qk_half(b, p, wi, qts, kts):
                qkv_qk_chunk(b, p, wi, 0, qts, kts)
                qkv_qk_chunk(b, p, wi, 1, qts, kts)

            def qkv_qk_pair(b, p, qts, kts, prologue=False):
                # ch-outer: q/k first halves land first (what scores j<4 need)
                for ch in range(2):
                    for wi in range(2):
                        # prologue: q copies on DVE, k on ACT (parallel gate)
                        ce = (None if wi == 0 else "act") if prologue else None
                        qkv_qk_chunk(b, p, wi, ch, qts, kts, cengine=ce)

            def qkv_v_tile(b, j, vts):
                # V for s-tile j: [128 s, 8 h, 64 d] -> vts[:, j, :, 0:64]
                ps = psB.tile([128, C], f32, tag="B", name=f"v_{b}_{j}")

                def stat(xp):
                    return lambda cc2: xts[xp][:, 2 * cc2:2 * cc2 + 2,
                                               b * T + j * 128: b * T + j * 128 + 128]

                def mov(w):
                    return lambda cc2: w[:, cc2 * 1024: (cc2 + 1) * 1024] \
                        .rearrange("p (two n) -> p two n", two=2)

                dr3(ps[:], stat("h"), stat("l"), mov(ws["wvh"]), mov(ws["wvl"]))
                nc.vector.tensor_copy(vts[:, j, :, 0:64],
                                      ps[:].rearrange("p (h d) -> p h d", h=8))

            def new_qkv_tiles(b):
                qts = qk_pool.tile([128, 4, T], bf16, tag="q", name=f"qts_{b}")
                kts = qk_pool.tile([128, 4, T], bf16, tag="k", name=f"kts_{b}")
                vts = v_pool.tile([128, 8, 8, 65], bf16, tag="v", name=f"vts_{b}")
                nc.gpsimd.memset(vts[:, :, :, 64:65], 1.0)
                return qts, kts, vts

            def att_scores(n, b, h, qts, kts):
                # scores + exp for head h; returns the P~ tile (bf16).
                al, p = h & 1, h >> 1
                pt = p_pool.tile([128, 8, T], bf16, tag="p", name=f"pt_{b}_{h}")
                for j in range(8):
                    sps = psA.tile([128, T], f32, tag="A", name=f"s_{b}_{h}_{j}")
                    for ch in range(2):
                        nc.tensor.matmul(
                            sps[:, ch * 512:(ch + 1) * 512],
                            kts[al * 64:al * 64 + 64, p, j * 128:j * 128 + 128],
                            qts[al * 64:al * 64 + 64, p, ch * 512:(ch + 1) * 512])
                    eng = s_eng[n * 4 + (j >> 1)]
                    if eng is None:
                        nc.scalar.activation(pt[:, j, :], sps[:], Exp, scale=EXP_SCALE)
                    else:
                        nc.vector.tensor_scalar(pt[:, j, :].bitcast(i16), sps[:],
                                                SCHRAU_A, SCHRAU_B, Mult, Add)
                return pt

            def att_pv(b, h, pt, vts, osb, rcp, eager=()):
                # PV in O-form with fused rowsum (col 64), two t-tile halves.
                # j reversed so the PV burst compresses after exp(h,7).
                for q in range(2):
                    oph = psB.tile([128, 4, HD + 1], f32, tag="B", name=f"o_{b}_{h}_{q}")
                    for tq in range(4):
                        tt = q * 4 + tq
                        for jj in range(8):
                            j = jj if q in eager else 7 - jj
                            nc.tensor.matmul(oph[:, tq, :],
                                             pt[:, j, tt * 128:tt * 128 + 128],
                                             vts[:, j, h, :],
                                             start=(jj == 0), stop=(jj == 7),
                                             skip_group_check=True)
                    nc.vector.reciprocal(rcp[:, q * 4:(q + 1) * 4, :],
                                         oph[:, :, 64:65])
                    nc.vector.tensor_tensor(
                        osb[:, q * 4:(q + 1) * 4, h * 64:h * 64 + 64],
                        oph[:, :, 0:64],
                        rcp[:, q * 4:(q + 1) * 4, :].to_broadcast([128, 4, 64]),
                        op=mybir.AluOpType.mult)

            def proj_tr(b, p, osb, ott):
                # O^T for hd-chunk p via the DMA XBAR transpose unit (8 tiles
                # of 16x128 per 128x128 block = 112ns DMA each; PE/DVE free)
                for tt in range(8):
                    nc.sync.dma_start_transpose(ott[:, p, tt * 128:(tt + 1) * 128],
                                                osb[:, tt, p * 128:(p + 1) * 128])

            def proj_y(b, tt, ott, pool=None):
                yps = (pool or psB).tile([128, C], f32,
                                         tag="A" if pool is psA else "B",
                                         name=f"y_{b}_{tt}")
                for p in range(4):
                    nc.tensor.matmul(yps[:],
                                     ott[:, p, tt * 128:tt * 128 + 128],
                                     wp_s[:, p * 512:(p + 1) * 512],
                                     start=(p == 0), stop=(p == 3))
                ysb = y_pool.tile([128, C], bf16, tag="y", name=f"ys_{b}_{tt}")
                nc.vector.tensor_add(ysb[:], yps[:], bias_b[:])
                nc.sync.dma_start(y[b * T + tt * 128: b * T + tt * 128 + 128, :].bitcast(bf16),
                                  ysb[:])

            # ---------------- emission schedule ----------------
            # Priority rule (priority == emission order): the exp-feeding
            # chain scores(n+3) outranks PV(n), which outranks filler work.
            q0, k0, v0 = new_qkv_tiles(0)
            osb0 = o_pool.tile([128, 8, C], bf16, tag="o", name="osb_0")
            q1, k1, v1 = new_qkv_tiles(1)
            osb1 = o_pool.tile([128, 8, C], bf16, tag="o", name="osb_1")
            ott0 = ot_pool.tile([128, 4, T], bf16, tag="ot", name="ott_0")
            ott1 = ot_pool.tile([128, 4, T], bf16, tag="ot", name="ott_1")

            pts = {}
            qkv_qk_pair(0, 0, q0, k0, prologue=True)
            pts[0] = att_scores(0, 0, 0, q0, k0)
            pts[1] = att_scores(1, 0, 1, q0, k0)
            qkv_qk_pair(0, 1, q0, k0)
            pts[2] = att_scores(2, 0, 2, q0, k0)
            for j in range(8):
                qkv_v_tile(0, j, v0)

            def filler(n):
                # n = global head index 0..15; the non-critical work wave.
                if n < 2:
                    qkv_qk_pair(0, 2 + n, q0, k0)
                elif 3 <= n < 11:
                    qkv_qk_half(1, (n - 3) >> 1, (n - 3) & 1, q1, k1)
                if n < 8:
                    qkv_v_tile(1, n, v1)
                if n in (2, 4, 6, 8):
                    proj_tr(0, (n - 2) // 2, osb0, ott0)
                if 9 <= n < 15:
                    proj_y(0, n - 9, ott0)
                if n == 12:
                    proj_y(0, 6, ott0)
                if n == 13:
                    proj_y(0, 7, ott0)
                if n in (10, 12):
                    proj_tr(1, (n - 10) // 2, osb1, ott1)
                if n == 13:
                    proj_tr(1, 2, osb1, ott1)

            for n in range(16):
                b, h = n >> 3, n & 7
                if n < 13:
                    nb, nh = (n + 3) >> 3, (n + 3) & 7
                    pts[n + 3] = att_scores(n + 3, nb, nh, q0 if nb == 0 else q1,
                                            k0 if nb == 0 else k1)
                rcp = r_pool.tile([128, 8, 1], f32, tag="rc", name=f"rcp_{n}")
                att_pv(b, h, pts.pop(n), v0 if b == 0 else v1,
                       osb0 if b == 0 else osb1, rcp,
                       eager=(0, 1) if n == 15 else ())
                filler(n)

            # batch 1 projection tail; scores pool (psA) is idle by now
            proj_tr(1, 3, osb1, ott1)
            for tt in range(8):
                proj_y(1, tt, ott1, pool=psB if tt % 2 else psA)

    nc.compile()
    return nc


def _pack_qk(w):
    # [NH, C, HD] -> [c, h*HD+d] -> tiled [c_local, p, cc, m] -> [128, 2048]
    wn = np.transpose(w, (1, 0, 2)).reshape(C, C)
    return np.ascontiguousarray(
        wn.reshape(4, 128, 4, 128).transpose(1, 2, 0, 3).reshape(128, 2048))


def _pack_cn(wn):
    # [C, N] natural -> tiled [c_local, cc, n] -> [128, 2048]
    return np.ascontiguousarray(wn.reshape(4, 128, C).transpose(1, 0, 2).reshape(128, 2048))


def get_nc():
    if "nc" not in _CACHE:
        _CACHE["nc"] = _build_nc()
    return _CACHE["nc"]


def make_in_maps(x, Wq, Wk, Wv, Wproj, bproj):
    x = np.asarray(x, dtype=np.float32)
    wqh, wql = _e4_split(_pack_qk(np.asarray(Wq, np.float32) * WSCALE))
    wkh, wkl = _e4_split(_pack_qk(np.asarray(Wk, np.float32) * WSCALE))
    wv_n = np.transpose(np.asarray(Wv, np.float32), (1, 0, 2)).reshape(C, C)
    wvh, wvl = _e4_split(_pack_cn(wv_n * WSCALE))
    wp_t = _bf16(_pack_cn(np.asarray(Wproj, np.float32) / WSCALE))
    bp_t = np.asarray(bproj, np.float32).reshape(1, C)
    in_maps = []
    for i in range(NCORES):
        xs = x[BL * i: BL * (i + 1)].reshape(BL, T, C)
        xt = np.ascontiguousarray(np.transpose(xs, (2, 0, 1)).reshape(C, BL * T))
        xth, xtl = _e4_split(xt)
        in_maps.append({
            "xh": xth, "xl": xtl,
            "wqh": wqh, "wql": wql, "wkh": wkh, "wkl": wkl,
            "wvh": wvh, "wvl": wvl, "wp": wp_t, "bp": bp_t,
        })
    return in_maps


def kernel(x, Wq, Wk, Wv, Wproj, bproj):
    from concourse.bass_utils import run_bass_kernel_spmd

    nc = get_nc()
    in_maps = make_in_maps(x, Wq, Wk, Wv, Wproj, bproj)
    trace = bool(int(os.environ.get("KERNEL_TRACE", "0")))
    res = run_bass_kernel_spmd(nc, in_maps, list(range(NCORES)), trace=trace)
    _CACHE["last_result"] = res
    out = np.empty((B, C, HH, WW), np.float32)
    for i in range(NCORES):
        yb = np.asarray(res.results[i]["y"]).view(np.uint16)
        yf = (yb.astype(np.uint32) << 16).view(np.float32)
        out[BL * i: BL * (i + 1)] = yf.reshape(BL, C, HH, WW)
    return out


# revision 8
# speedup vs baseline: 1.0308x; 1.0308x over previous
"""Multi-head attention Trainium2 kernel (8 NeuronCores, data-parallel over batch).

Per-core program (2 batches per core), optimized for the TimelineSim cost
model (matmul charged = out_free_size x cycles_per_row; M and K are free;
fp8 DoubleRow = 0.5 cycles/row over 2 K-chunks; every instruction carries
tens-of-ns sequencer/semaphore overhead).

Structure:
  x arrives split hi/lo fp8e4m3 (x = xh + xl exactly at bf16-grade precision)
  -> QKV projections via fp8 DoubleRow in 3 passes (xh*Wh + xh*Wl + xl*Wh):
     1536 rows per 512-wide chunk group vs 2048 bf16 (-25% PE).  Q/K/V
     weights are host-scaled by 16 so the fp8 split clears e4m3 subnormals;
     the 256x on scores folds into the exp scale, the 16x on V folds into
     Wproj (host-divided).
  -> scores S^T [s, t] per (head, s-tile): ONE bf16 matmul [128, 1024]
  -> exp split across three engines (ACT exact; DVE/Pool via bf16
     Schraudolph: one fused tensor_scalar mult+add -> int16, bitcast bf16;
     ~10/128 tiles each) so the ACT stream ends before PE and the tail
     projection never stalls the span.
  -> PV in O-form with fused rowsum column (bf16), reversed-j accumulation
  -> normalize via DVE reciprocal + broadcast multiply -> osb bf16
  -> O^T via bf16 PE transposes -> output projection (Wproj/16) + bias
PE is the binding engine (~131us); ACT ~112us, DVE ~46us, Pool ~48us.
"""
import sys
import os

sys.path.insert(0, "/opt/trn_rl_repo")
import numpy as np
import ml_dtypes

B, C, HH, WW = 16, 512, 32, 32
T = HH * WW              # 1024
NH, HD = 8, 64
BL = 2                   # batches per core
NCORES = 8

WSCALE = 16.0            # host scale on Wq/Wk/Wv for fp8 hi/lo split
EXP_SCALE = 0.125 / (WSCALE * WSCALE)          # folds q,k scaling into exp
SCHRAU_A = EXP_SCALE * 128.0 / float(np.log(2.0))
SCHRAU_B = 16256.0 - 7.0                       # bf16 one bias + centering adj

N_S_PAIRS = 6            # of 64 j-pair units, computed via DVE Schraudolph

_CACHE = {}


def _bf16(a):
    """f32 -> bf16 bits (round to nearest even), as uint16."""
    u = np.ascontiguousarray(a, dtype=np.float32).view(np.uint32)
    r = (u + 0x7FFF + ((u >> 16) & 1)) >> 16
    return r.astype(np.uint16)


def _e4_split(a):
    """f32 -> (hi, lo) float8_e4m3 byte arrays with a == hi + lo approx."""
    hi = a.astype(ml_dtypes.float8_e4m3)
    lo = (a - hi.astype(np.float32)).astype(ml_dtypes.float8_e4m3)
    return hi.view(np.uint8), lo.view(np.uint8)


def _s_pair_engines():
    """Map pair unit u (0..63) -> None (ACT) or alternating 'dve'/'pool'."""
    out = {}
    s_count = 0
    for u in range(64):
        if (u * N_S_PAIRS) // 64 != ((u + 1) * N_S_PAIRS) // 64:
            out[u] = ("dve", "dve")
            s_count += 1
        else:
            out[u] = None
    return out


def _build_nc():
    import concourse.bacc as bacc
    import concourse.mybir as mybir
    import concourse.tile as tile
    from concourse import masks

    f32 = mybir.dt.float32
    bf16 = mybir.dt.bfloat16
    u16 = mybir.dt.uint16
    u8 = mybir.dt.uint8
    i16 = mybir.dt.int16
    e4 = mybir.dt.float8e4
    Exp = mybir.ActivationFunctionType.Exp
    DR = mybir.MatmulPerfMode.DoubleRow
    Mult = mybir.AluOpType.mult
    Add = mybir.AluOpType.add

    nc = bacc.Bacc("TRN2", target_bir_lowering=False, debug=False, num_devices=NCORES)
    xh_d = nc.dram_tensor("xh", [C, BL * T], u8, kind="ExternalInput").ap()
    xl_d = nc.dram_tensor("xl", [C, BL * T], u8, kind="ExternalInput").ap()
    w_d = {}
    for nm in ("wqh", "wql", "wkh", "wkl", "wvh", "wvl"):
        w_d[nm] = nc.dram_tensor(nm, [128, 2048], u8, kind="ExternalInput").ap()
    wp = nc.dram_tensor("wp", [128, 2048], u16, kind="ExternalInput").ap()
    bp = nc.dram_tensor("bp", [1, C], f32, kind="ExternalInput").ap()
    y = nc.dram_tensor("y", [BL * T, C], u16, kind="ExternalOutput").ap()

    s_eng = _s_pair_engines()

    with tile.TileContext(nc) as tc:
        with tc.tile_pool(name="const", bufs=1) as cpool, \
             tc.tile_pool(name="qk", bufs=2) as qk_pool, \
             tc.tile_pool(name="vv", bufs=2) as v_pool, \
             tc.tile_pool(name="pp", bufs=4) as p_pool, \
             tc.tile_pool(name="ob", bufs=2) as o_pool, \
             tc.tile_pool(name="ot", bufs=2) as ot_pool, \
             tc.tile_pool(name="yy", bufs=8) as y_pool, \
             tc.tile_pool(name="rr", bufs=3) as r_pool, \
             tc.tile_pool(name="psA", bufs=3, space="PSUM") as psA, \
             tc.tile_pool(name="psB", bufs=2, space="PSUM") as psB:

            # ---- constants + weights ----
            xts = {}
            for pname in ("h", "l"):
                xts[pname] = cpool.tile([128, 4, BL * T], e4, tag=f"xt{pname}",
                                        name=f"xts_{pname}")
            xsrc = {"h": xh_d.bitcast(e4).rearrange("(cc p) t -> p cc t", cc=4),
                    "l": xl_d.bitcast(e4).rearrange("(cc p) t -> p cc t", cc=4)}
            ws = {nm: cpool.tile([128, 2048], e4, tag=nm, name=f"ws_{nm}")
                  for nm in ("wqh", "wql", "wkh", "wkl", "wvh", "wvl")}
            wp_s = cpool.tile([128, 2048], bf16, tag="wp")
            bias_b = cpool.tile([128, C], f32, tag="bias")
            ident = cpool.tile([128, 128], bf16, tag="ident")
            masks.make_identity(nc, ident[:])

            # ACT: trigger the Exp table load immediately (before any exp
            # work is possible) with a 1-element dummy activation.
            dummy = cpool.tile([128, 1], f32, tag="dummy")
            nc.scalar.activation(dummy[:], ident[:, 0:1], Exp, scale=1.0)

            # Prologue DMAs. Gating chain for the first scores: wq{h,l}/wk{h,l}
            # pair-0 cols + x{h,l} t 0:1024 -- big chunks on SP (fewest 565ns
            # issues), k-weights on ACT, all bulk transfers via Pool SWDGE
            # (idle engine, ~1.1us each, not latency-critical).
            nc.sync.dma_start(ws["wqh"][:, 0:512], w_d["wqh"].bitcast(e4)[:, 0:512])
            nc.scalar.dma_start(ws["wkh"][:, 0:512], w_d["wkh"].bitcast(e4)[:, 0:512])
            nc.sync.dma_start(xts["h"][:, :, 0:1024], xsrc["h"][:, :, 0:1024])
            nc.scalar.dma_start(ws["wkl"][:, 0:512], w_d["wkl"].bitcast(e4)[:, 0:512])
            nc.sync.dma_start(xts["l"][:, :, 0:1024], xsrc["l"][:, :, 0:1024])
            nc.sync.dma_start(ws["wql"][:, 0:512], w_d["wql"].bitcast(e4)[:, 0:512])
            nc.sync.dma_start(ws["wvh"][:], w_d["wvh"].bitcast(e4))
            nc.sync.dma_start(ws["wvl"][:], w_d["wvl"].bitcast(e4))
            for nm in ("wqh", "wkh", "wql", "wkl"):
                nc.gpsimd.dma_start(ws[nm][:, 512:2048], w_d[nm].bitcast(e4)[:, 512:2048])
            for pname in ("h", "l"):
                nc.gpsimd.dma_start(xts[pname][:, :, 1024:2048],
                                    xsrc[pname][:, :, 1024:2048])
            nc.gpsimd.dma_start(wp_s[:], wp.bitcast(bf16))
            nc.gpsimd.dma_start(bias_b[:], bp.to_broadcast([128, C]))

            # ---- PE warmup: ramp the pstate clock during the DMA prologue
            wps = psB.tile([128, 128], bf16, tag="B", name="warm")
            for i in range(14):
                nc.tensor.transpose(wps[:], ident[:], ident[:])

            def dr3(ps, stat_hi, stat_lo, mov_hi, mov_lo):
                """3-pass hi/lo fp8 DoubleRow accumulation group.
                stat/mov are fns: cc2 -> AP ([128, 2, m] / [128, 2, n])."""
                passes = [(stat_hi, mov_hi), (stat_lo, mov_hi), (stat_hi, mov_lo)]
                n = 0
                for sf, mf in passes:
                    for cc2 in range(2):
                        nc.tensor.matmul(ps, sf(cc2), mf(cc2),
                                         start=(n == 0), stop=(n == 5),
                                         perf_mode=DR)
                        n += 1

            def qkv_qk_chunk(b, p, wi, ch, qts, kts):
                # one 512-wide t-chunk of Q^T or K^T for head-pair p.
                whi, wlo = (ws["wqh"], ws["wql"]) if wi == 0 else (ws["wkh"], ws["wkl"])
                dst = qts if wi == 0 else kts
                ps = psB.tile([128, 512], f32, tag="B", name=f"qk_{b}_{p}_{wi}_{ch}")
                t0 = b * T + ch * 512

                def stat(w):
                    return lambda cc2: w[:, p * 512 + cc2 * 256: p * 512 + cc2 * 256 + 256] \
                        .rearrange("p (two m) -> p two m", two=2)

                def mov(xp):
                    return lambda cc2: xts[xp][:, 2 * cc2:2 * cc2 + 2, t0:t0 + 512]

                dr3(ps[:], stat(whi), stat(wlo), mov("h"), mov("l"))
                nc.vector.tensor_copy(dst[:, p, ch * 512:(ch + 1) * 512], ps[:])

            def qkv_qk_half(b, p, wi, qts, kts):
                qkv_qk_chunk(b, p, wi, 0, qts, kts)
                qkv_qk_chunk(b, p, wi, 1, qts, kts)

            def qkv_qk_pair(b, p, qts, kts, prologue=False):
                if prologue:
                    # dependency order of the first exp: k-ch0, q-ch0, q-ch1
                    # gate scores j<4; k-ch1 last.  All copies on DVE.
                    for wi, ch in ((1, 0), (0, 0), (0, 1), (1, 1)):
                        qkv_qk_chunk(b, p, wi, ch, qts, kts)
                    return
                # ch-outer: q/k first halves land first (what scores j<4 need)
                for ch in range(2):
                    for wi in range(2):
                        qkv_qk_chunk(b, p, wi, ch, qts, kts)

            def qkv_v_tile(b, j, vts):
                # V for s-tile j: [128 s, 8 h, 64 d] -> vts[:, j, :, 0:64]
                ps = psB.tile([128, C], f32, tag="B", name=f"v_{b}_{j}")

                def stat(xp):
                    return lambda cc2: xts[xp][:, 2 * cc2:2 * cc2 + 2,
                                               b * T + j * 128: b * T + j * 128 + 128]

                def mov(w):
                    return lambda cc2: w[:, cc2 * 1024: (cc2 + 1) * 1024] \
                        .rearrange("p (two n) -> p two n", two=2)

                dr3(ps[:], stat("h"), stat("l"), mov(ws["wvh"]), mov(ws["wvl"]))
                nc.vector.tensor_copy(vts[:, j, :, 0:64],
                                      ps[:].rearrange("p (h d) -> p h d", h=8))

            def new_qkv_tiles(b):
                qts = qk_pool.tile([128, 4, T], bf16, tag="q", name=f"qts_{b}")
                kts = qk_pool.tile([128, 4, T], bf16, tag="k", name=f"kts_{b}")
                vts = v_pool.tile([128, 8, 8, 65], bf16, tag="v", name=f"vts_{b}")
                nc.gpsimd.memset(vts[:, :, :, 64:65], 1.0)
                return qts, kts, vts

            def att_scores(n, b, h, qts, kts):
                # scores + exp for head h; returns the P~ tile (bf16).
                al, p = h & 1, h >> 1
                pt = p_pool.tile([128, 8, T], bf16, tag="p", name=f"pt_{b}_{h}")
                for j in range(8):
                    sps = psA.tile([128, T], f32, tag="A", name=f"s_{b}_{h}_{j}")
                    for ch in range(2):
                        nc.tensor.matmul(
                            sps[:, ch * 512:(ch + 1) * 512],
                            kts[al * 64:al * 64 + 64, p, j * 128:j * 128 + 128],
                            qts[al * 64:al * 64 + 64, p, ch * 512:(ch + 1) * 512])
                    eng = s_eng[n * 4 + (j >> 1)]
                    if eng is None:
                        nc.scalar.activation(pt[:, j, :], sps[:], Exp, scale=EXP_SCALE)
                    else:
                        nc.vector.tensor_scalar(pt[:, j, :].bitcast(i16), sps[:],
                                                SCHRAU_A, SCHRAU_B, Mult, Add)
                return pt

            def att_pv(b, h, pt, vts, osb, rcp, eager=()):
                # PV in O-form with fused rowsum (col 64), two t-tile halves.
                # j reversed so the PV burst compresses after exp(h,7).
                for q in range(2):
                    oph = psB.tile([128, 4, HD + 1], f32, tag="B", name=f"o_{b}_{h}_{q}")
                    for tq in range(4):
                        tt = q * 4 + tq
                        for jj in range(8):
                            j = jj if q in eager else 7 - jj
                            nc.tensor.matmul(oph[:, tq, :],
                                             pt[:, j, tt * 128:tt * 128 + 128],
                                             vts[:, j, h, :],
                                             start=(jj == 0), stop=(jj == 7),
                                             skip_group_check=True)
                    nc.vector.reciprocal(rcp[:, q * 4:(q + 1) * 4, :],
                                         oph[:, :, 64:65])
                    nc.vector.tensor_tensor(
                        osb[:, q * 4:(q + 1) * 4, h * 64:h * 64 + 64],
                        oph[:, :, 0:64],
                        rcp[:, q * 4:(q + 1) * 4, :].to_broadcast([128, 4, 64]),
                        op=mybir.AluOpType.mult)

            def proj_tr(b, p, osb, ott, on_pe=False):
                # O^T for hd-chunk p.  Mid-stream chunks ride the DMA XBAR
                # transpose unit (112ns DMA per 128x128 block, PE/DVE free);
                # the tail-critical last chunk uses PE transposes (the DMA
                # path's 8x(565 issue + 625 HWDGE) would stall the tail).
                if on_pe:
                    tps = psB.tile([128, T], bf16, tag="B", name=f"tps_{b}_{p}")
                    for tt in range(8):
                        nc.tensor.transpose(tps[:, tt * 128:tt * 128 + 128],
                                            osb[:, tt, p * 128:(p + 1) * 128], ident[:])
                    nc.vector.tensor_copy(ott[:, p, :], tps[:])
                    return
                for tt in range(8):
                    nc.sync.dma_start_transpose(ott[:, p, tt * 128:(tt + 1) * 128],
                                                osb[:, tt, p * 128:(p + 1) * 128])

            def proj_y(b, tt, ott, pool=None):
                yps = (pool or psB).tile([128, C], f32,
                                         tag="A" if pool is psA else "B",
                                         name=f"y_{b}_{tt}")
                for p in range(4):
                    nc.tensor.matmul(yps[:],
                                     ott[:, p, tt * 128:tt * 128 + 128],
                                     wp_s[:, p * 512:(p + 1) * 512],
                                     start=(p == 0), stop=(p == 3))
                ysb = y_pool.tile([128, C], bf16, tag="y", name=f"ys_{b}_{tt}")
                nc.vector.tensor_add(ysb[:], yps[:], bias_b[:])
                nc.sync.dma_start(y[b * T + tt * 128: b * T + tt * 128 + 128, :].bitcast(bf16),
                                  ysb[:])

            # ---------------- emission schedule ----------------
            # Priority rule (priority == emission order): the exp-feeding
            # chain scores(n+3) outranks PV(n), which outranks filler work.
            q0, k0, v0 = new_qkv_tiles(0)
            osb0 = o_pool.tile([128, 8, C], bf16, tag="o", name="osb_0")
            q1, k1, v1 = new_qkv_tiles(1)
            osb1 = o_pool.tile([128, 8, C], bf16, tag="o", name="osb_1")
            ott0 = ot_pool.tile([128, 4, T], bf16, tag="ot", name="ott_0")
            ott1 = ot_pool.tile([128, 4, T], bf16, tag="ot", name="ott_1")

            pts = {}
            qkv_qk_pair(0, 0, q0, k0, prologue=True)
            pts[0] = att_scores(0, 0, 0, q0, k0)
            pts[1] = att_scores(1, 0, 1, q0, k0)
            qkv_qk_pair(0, 1, q0, k0)
            pts[2] = att_scores(2, 0, 2, q0, k0)
            for j in range(8):
                qkv_v_tile(0, j, v0)

            def filler(n):
                # n = global head index 0..15; the non-critical work wave.
                if n < 2:
                    qkv_qk_pair(0, 2 + n, q0, k0)
                elif 3 <= n < 11:
                    qkv_qk_half(1, (n - 3) >> 1, (n - 3) & 1, q1, k1)
                if n < 8:
                    qkv_v_tile(1, n, v1)
                if n in (2, 4, 6, 8):
                    proj_tr(0, (n - 2) // 2, osb0, ott0)
                if 9 <= n < 15:
                    proj_y(0, n - 9, ott0)
                if n == 12:
                    proj_y(0, 6, ott0)
                if n == 13:
                    proj_y(0, 7, ott0)
                if n in (10, 12):
                    proj_tr(1, (n - 10) // 2, osb1, ott1)
                if n == 13:
                    proj_tr(1, 2, osb1, ott1)

            for n in range(16):
                b, h = n >> 3, n & 7
                if n < 13:
                    nb, nh = (n + 3) >> 3, (n + 3) & 7
                    pts[n + 3] = att_scores(n + 3, nb, nh, q0 if nb == 0 else q1,
                                            k0 if nb == 0 else k1)
                rcp = r_pool.tile([128, 8, 1], f32, tag="rc", name=f"rcp_{n}")
                att_pv(b, h, pts.pop(n), v0 if b == 0 else v1,
                       osb0 if b == 0 else osb1, rcp,
                       eager=(0, 1) if n == 15 else ())
                filler(n)

            # batch 1 projection tail; scores pool (psA) is idle by now
            proj_tr(1, 3, osb1, ott1, on_pe=True)
            for tt in range(8):
                proj_y(1, tt, ott1, pool=psB if tt % 2 else psA)

    nc.compile()
    return nc


def _pack_qk(w):
    # [NH, C, HD] -> [c, h*HD+d] -> tiled [c_local, p, cc, m] -> [128, 2048]
    wn = np.transpose(w, (1, 0, 2)).reshape(C, C)
    return np.ascontiguousarray(
        wn.reshape(4, 128, 4, 128).transpose(1, 2, 0, 3).reshape(128, 2048))


def _pack_cn(wn):
    # [C, N] natural -> tiled [c_local, cc, n] -> [128, 2048]
    return np.ascontiguousarray(wn.reshape(4, 128, C).transpose(1, 0, 2).reshape(128, 2048))


def get_nc():
    if "nc" not in _CACHE:
        _CACHE["nc"] = _build_nc()
    return _CACHE["nc"]


def make_in_maps(x, Wq, Wk, Wv, Wproj, bproj):
    x = np.asarray(x, dtype=np.float32)
    wqh, wql = _e4_split(_pack_qk(np.asarray(Wq, np.float32) * WSCALE))
    wkh, wkl = _e4_split(_pack_qk(np.asarray(Wk, np.float32) * WSCALE))
    wv_n = np.transpose(np.asarray(Wv, np.float32), (1, 0, 2)).reshape(C, C)
    wvh, wvl = _e4_split(_pack_cn(wv_n * WSCALE))
    wp_t = _bf16(_pack_cn(np.asarray(Wproj, np.float32) / WSCALE))
    bp_t = np.asarray(bproj, np.float32).reshape(1, C)
    in_maps = []
    for i in range(NCORES):
        xs = x[BL * i: BL * (i + 1)].reshape(BL, T, C)
        xt = np.ascontiguousarray(np.transpose(xs, (2, 0, 1)).reshape(C, BL * T))
        xth, xtl = _e4_split(xt)
        in_maps.append({
            "xh": xth, "xl": xtl,
            "wqh": wqh, "wql": wql, "wkh": wkh, "wkl": wkl,
            "wvh": wvh, "wvl": wvl, "wp": wp_t, "bp": bp_t,
        })
    return in_maps


def kernel(x, Wq, Wk, Wv, Wproj, bproj):
    from concourse.bass_utils import run_bass_kernel_spmd

    nc = get_nc()
    in_maps = make_in_maps(x, Wq, Wk, Wv, Wproj, bproj)
    trace = bool(int(os.environ.get("KERNEL_TRACE", "0")))
    res = run_bass_kernel_spmd(nc, in_maps, list(range(NCORES)), trace=trace)
    _CACHE["last_result"] = res
    out = np.empty((B, C, HH, WW), np.float32)
    for i in range(NCORES):
        yb = np.asarray(res.results[i]["y"]).view(np.uint16)
        yf = (yb.astype(np.uint32) << 16).view(np.float32)
        out[BL * i: BL * (i + 1)] = yf.reshape(BL, C, HH, WW)
    return out


# revision 9
# speedup vs baseline: 1.0467x; 1.0154x over previous
"""Multi-head attention Trainium2 kernel (8 NeuronCores, data-parallel over batch).

Per-core program (2 batches per core), optimized for the TimelineSim cost
model (matmul charged = out_free_size x cycles_per_row; M and K are free;
fp8 DoubleRow = 0.5 cycles/row over 2 K-chunks; every instruction carries
tens-of-ns sequencer/semaphore overhead).

Structure:
  x arrives split hi/lo fp8e4m3 (x = xh + xl exactly at bf16-grade precision)
  -> QKV projections via fp8 DoubleRow in 3 passes (xh*Wh + xh*Wl + xl*Wh):
     1536 rows per 512-wide chunk group vs 2048 bf16 (-25% PE).  Q/K/V
     weights are host-scaled by 16 so the fp8 split clears e4m3 subnormals;
     the 256x on scores folds into the exp scale, the 16x on V folds into
     Wproj (host-divided).
  -> scores S^T [s, t] per (head, s-tile): ONE bf16 matmul [128, 1024]
  -> exp split across three engines (ACT exact; DVE/Pool via bf16
     Schraudolph: one fused tensor_scalar mult+add -> int16, bitcast bf16;
     ~10/128 tiles each) so the ACT stream ends before PE and the tail
     projection never stalls the span.
  -> PV in O-form with fused rowsum column (bf16), reversed-j accumulation
  -> normalize via DVE reciprocal + broadcast multiply -> osb bf16
  -> O^T via bf16 PE transposes -> output projection (Wproj/16) + bias
PE is the binding engine (~131us); ACT ~112us, DVE ~46us, Pool ~48us.
"""
import sys
import os

sys.path.insert(0, "/opt/trn_rl_repo")
import numpy as np
import ml_dtypes

B, C, HH, WW = 16, 512, 32, 32
T = HH * WW              # 1024
NH, HD = 8, 64
BL = 2                   # batches per core
NCORES = 8

WSCALE = 16.0            # host scale on Wq/Wk/Wv for fp8 hi/lo split
EXP_SCALE = 0.125 / (WSCALE * WSCALE)          # folds q,k scaling into exp
SCHRAU_A = EXP_SCALE * 128.0 / float(np.log(2.0))
SCHRAU_B = 16256.0 - 7.0                       # bf16 one bias + centering adj

N_S_PAIRS = 10           # of 64 j-pair units, computed via DVE Schraudolph

_CACHE = {}


def _bf16(a):
    """f32 -> bf16 bits (round to nearest even), as uint16."""
    u = np.ascontiguousarray(a, dtype=np.float32).view(np.uint32)
    r = (u + 0x7FFF + ((u >> 16) & 1)) >> 16
    return r.astype(np.uint16)


def _e4_split(a):
    """f32 -> (hi, lo) float8_e4m3 byte arrays with a == hi + lo approx."""
    hi = a.astype(ml_dtypes.float8_e4m3)
    lo = (a - hi.astype(np.float32)).astype(ml_dtypes.float8_e4m3)
    return hi.view(np.uint8), lo.view(np.uint8)


def _s_pair_engines():
    """Map pair unit u (0..63) -> None (ACT) or alternating 'dve'/'pool'."""
    out = {}
    s_count = 0
    for u in range(64):
        if (u * N_S_PAIRS) // 64 != ((u + 1) * N_S_PAIRS) // 64:
            out[u] = ("dve", "dve")
            s_count += 1
        else:
            out[u] = None
    return out


def _build_nc():
    import concourse.bacc as bacc
    import concourse.mybir as mybir
    import concourse.tile as tile
    from concourse import masks

    f32 = mybir.dt.float32
    bf16 = mybir.dt.bfloat16
    u16 = mybir.dt.uint16
    u8 = mybir.dt.uint8
    i16 = mybir.dt.int16
    e4 = mybir.dt.float8e4
    Exp = mybir.ActivationFunctionType.Exp
    DR = mybir.MatmulPerfMode.DoubleRow
    Mult = mybir.AluOpType.mult
    Add = mybir.AluOpType.add

    nc = bacc.Bacc("TRN2", target_bir_lowering=False, debug=False, num_devices=NCORES)
    xh_d = nc.dram_tensor("xh", [C, BL * T], u8, kind="ExternalInput").ap()
    xl_d = nc.dram_tensor("xl", [C, BL * T], u8, kind="ExternalInput").ap()
    w_d = {}
    for nm in ("wqh", "wql", "wkh", "wkl", "wvh", "wvl"):
        w_d[nm] = nc.dram_tensor(nm, [128, 2048], u8, kind="ExternalInput").ap()
    wp = nc.dram_tensor("wp", [128, 2048], u16, kind="ExternalInput").ap()
    bp = nc.dram_tensor("bp", [1, C], f32, kind="ExternalInput").ap()
    y = nc.dram_tensor("y", [BL * T, C], u16, kind="ExternalOutput").ap()

    s_eng = _s_pair_engines()

    with tile.TileContext(nc) as tc:
        with tc.tile_pool(name="const", bufs=1) as cpool, \
             tc.tile_pool(name="qk", bufs=2) as qk_pool, \
             tc.tile_pool(name="vv", bufs=2) as v_pool, \
             tc.tile_pool(name="pp", bufs=4) as p_pool, \
             tc.tile_pool(name="ob", bufs=2) as o_pool, \
             tc.tile_pool(name="ot", bufs=2) as ot_pool, \
             tc.tile_pool(name="yy", bufs=8) as y_pool, \
             tc.tile_pool(name="rr", bufs=3) as r_pool, \
             tc.tile_pool(name="psA", bufs=3, space="PSUM") as psA, \
             tc.tile_pool(name="psB", bufs=2, space="PSUM") as psB:

            # ---- constants + weights ----
            xts = {}
            for pname in ("h", "l"):
                xts[pname] = cpool.tile([128, 4, BL * T], e4, tag=f"xt{pname}",
                                        name=f"xts_{pname}")
            xsrc = {"h": xh_d.bitcast(e4).rearrange("(cc p) t -> p cc t", cc=4),
                    "l": xl_d.bitcast(e4).rearrange("(cc p) t -> p cc t", cc=4)}
            ws = {nm: cpool.tile([128, 2048], e4, tag=nm, name=f"ws_{nm}")
                  for nm in ("wqh", "wql", "wkh", "wkl", "wvh", "wvl")}
            wp_s = cpool.tile([128, 2048], bf16, tag="wp")
            bias_b = cpool.tile([128, C], f32, tag="bias")
            ident = cpool.tile([128, 128], bf16, tag="ident")
            masks.make_identity(nc, ident[:])

            # ACT: trigger the Exp table load immediately (before any exp
            # work is possible) with a 1-element dummy activation.
            dummy = cpool.tile([128, 1], f32, tag="dummy")
            nc.scalar.activation(dummy[:], ident[:, 0:1], Exp, scale=1.0)

            # Prologue DMAs. Gating chain for the first scores: wq{h,l}/wk{h,l}
            # pair-0 cols + x{h,l} t 0:1024 -- big chunks on SP (fewest 565ns
            # issues), k-weights on ACT, all bulk transfers via Pool SWDGE
            # (idle engine, ~1.1us each, not latency-critical).
            nc.sync.dma_start(ws["wqh"][:, 0:512], w_d["wqh"].bitcast(e4)[:, 0:512])
            nc.scalar.dma_start(ws["wkh"][:, 0:512], w_d["wkh"].bitcast(e4)[:, 0:512])
            nc.sync.dma_start(xts["h"][:, 0:2, 0:1024], xsrc["h"][:, 0:2, 0:1024])
            nc.scalar.dma_start(ws["wkl"][:, 0:512], w_d["wkl"].bitcast(e4)[:, 0:512])
            nc.sync.dma_start(xts["h"][:, 2:4, 0:1024], xsrc["h"][:, 2:4, 0:1024])
            nc.scalar.dma_start(ws["wql"][:, 0:512], w_d["wql"].bitcast(e4)[:, 0:512])
            nc.sync.dma_start(xts["l"][:, 0:2, 0:1024], xsrc["l"][:, 0:2, 0:1024])
            nc.sync.dma_start(xts["l"][:, 2:4, 0:1024], xsrc["l"][:, 2:4, 0:1024])
            nc.sync.dma_start(ws["wvh"][:], w_d["wvh"].bitcast(e4))
            nc.sync.dma_start(ws["wvl"][:], w_d["wvl"].bitcast(e4))
            for nm in ("wqh", "wkh", "wql", "wkl"):
                nc.sync.dma_start(ws[nm][:, 512:2048], w_d[nm].bitcast(e4)[:, 512:2048])
            for pname in ("h", "l"):
                nc.sync.dma_start(xts[pname][:, :, 1024:2048],
                                  xsrc[pname][:, :, 1024:2048])
            nc.sync.dma_start(wp_s[:], wp.bitcast(bf16))
            nc.sync.dma_start(bias_b[:], bp.to_broadcast([128, C]))

            # ---- PE warmup: ramp the pstate clock during the DMA prologue
            wps = psB.tile([128, 128], bf16, tag="B", name="warm")
            for i in range(14):
                nc.tensor.transpose(wps[:], ident[:], ident[:])

            def dr3(ps, stat_hi, stat_lo, mov_hi, mov_lo):
                """3-pass hi/lo fp8 DoubleRow accumulation group.
                stat/mov are fns: cc2 -> AP ([128, 2, m] / [128, 2, n])."""
                passes = [(stat_hi, mov_hi), (stat_lo, mov_hi), (stat_hi, mov_lo)]
                n = 0
                for sf, mf in passes:
                    for cc2 in range(2):
                        nc.tensor.matmul(ps, sf(cc2), mf(cc2),
                                         start=(n == 0), stop=(n == 5),
                                         perf_mode=DR)
                        n += 1

            def qkv_qk_chunk(b, p, wi, ch, qts, kts, on_act=False):
                # one 512-wide t-chunk of Q^T or K^T for head-pair p.
                whi, wlo = (ws["wqh"], ws["wql"]) if wi == 0 else (ws["wkh"], ws["wkl"])
                dst = qts if wi == 0 else kts
                ps = psB.tile([128, 512], f32, tag="B", name=f"qk_{b}_{p}_{wi}_{ch}")
                t0 = b * T + ch * 512

                def stat(w):
                    return lambda cc2: w[:, p * 512 + cc2 * 256: p * 512 + cc2 * 256 + 256] \
                        .rearrange("p (two m) -> p two m", two=2)

                def mov(xp):
                    return lambda cc2: xts[xp][:, 2 * cc2:2 * cc2 + 2, t0:t0 + 512]

                dr3(ps[:], stat(whi), stat(wlo), mov("h"), mov("l"))
                if on_act:
                    nc.scalar.copy(dst[:, p, ch * 512:(ch + 1) * 512], ps[:])
                else:
                    nc.vector.tensor_copy(dst[:, p, ch * 512:(ch + 1) * 512], ps[:])

            def qkv_qk_half(b, p, wi, qts, kts):
                qkv_qk_chunk(b, p, wi, 0, qts, kts)
                qkv_qk_chunk(b, p, wi, 1, qts, kts)

            def qkv_qk_pair(b, p, qts, kts, prologue=False):
                if prologue:
                    # dependency order of the first exp: k-ch0 (copy on the
                    # still-idle ACT), q-ch0, q-ch1 on DVE; k-ch1 last.
                    qkv_qk_chunk(b, p, 1, 0, qts, kts, on_act=True)
                    qkv_qk_chunk(b, p, 0, 0, qts, kts)
                    qkv_qk_chunk(b, p, 0, 1, qts, kts)
                    qkv_qk_chunk(b, p, 1, 1, qts, kts)
                    return
                # ch-outer: q/k first halves land first (what scores j<4 need)
                for ch in range(2):
                    for wi in range(2):
                        qkv_qk_chunk(b, p, wi, ch, qts, kts)

            def qkv_v_tile(b, j, vts):
                # V for s-tile j: [128 s, 8 h, 64 d] -> vts[:, j, :, 0:64]
                ps = psB.tile([128, C], f32, tag="B", name=f"v_{b}_{j}")

                def stat(xp):
                    return lambda cc2: xts[xp][:, 2 * cc2:2 * cc2 + 2,
                                               b * T + j * 128: b * T + j * 128 + 128]

                def mov(w):
                    return lambda cc2: w[:, cc2 * 1024: (cc2 + 1) * 1024] \
                        .rearrange("p (two n) -> p two n", two=2)

                dr3(ps[:], stat("h"), stat("l"), mov(ws["wvh"]), mov(ws["wvl"]))
                nc.vector.tensor_copy(vts[:, j, :, 0:64],
                                      ps[:].rearrange("p (h d) -> p h d", h=8))

            def new_qkv_tiles(b):
                qts = qk_pool.tile([128, 4, T], bf16, tag="q", name=f"qts_{b}")
                kts = qk_pool.tile([128, 4, T], bf16, tag="k", name=f"kts_{b}")
                vts = v_pool.tile([128, 8, 8, 65], bf16, tag="v", name=f"vts_{b}")
                nc.gpsimd.memset(vts[:, :, :, 64:65], 1.0)
                return qts, kts, vts

            def att_scores(n, b, h, qts, kts):
                # scores + exp for head h; returns the P~ tile (bf16).
                al, p = h & 1, h >> 1
                pt = p_pool.tile([128, 8, T], bf16, tag="p", name=f"pt_{b}_{h}")
                for j in range(8):
                    sps = psA.tile([128, T], f32, tag="A", name=f"s_{b}_{h}_{j}")
                    for ch in range(2):
                        nc.tensor.matmul(
                            sps[:, ch * 512:(ch + 1) * 512],
                            kts[al * 64:al * 64 + 64, p, j * 128:j * 128 + 128],
                            qts[al * 64:al * 64 + 64, p, ch * 512:(ch + 1) * 512])
                    eng = s_eng[n * 4 + (j >> 1)]
                    if eng is None:
                        nc.scalar.activation(pt[:, j, :], sps[:], Exp, scale=EXP_SCALE)
                    else:
                        nc.vector.tensor_scalar(pt[:, j, :].bitcast(i16), sps[:],
                                                SCHRAU_A, SCHRAU_B, Mult, Add)
                return pt

            def att_pv(b, h, pt, vts, osb, rcp, eager=()):
                # PV in O-form with fused rowsum (col 64), two t-tile halves.
                # j reversed so the PV burst compresses after exp(h,7).
                for q in range(2):
                    oph = psB.tile([128, 4, HD + 1], f32, tag="B", name=f"o_{b}_{h}_{q}")
                    for tq in range(4):
                        tt = q * 4 + tq
                        for jj in range(8):
                            j = jj if q in eager else 7 - jj
                            nc.tensor.matmul(oph[:, tq, :],
                                             pt[:, j, tt * 128:tt * 128 + 128],
                                             vts[:, j, h, :],
                                             start=(jj == 0), stop=(jj == 7),
                                             skip_group_check=True)
                    nc.vector.reciprocal(rcp[:, q * 4:(q + 1) * 4, :],
                                         oph[:, :, 64:65])
                    nc.vector.tensor_tensor(
                        osb[:, q * 4:(q + 1) * 4, h * 64:h * 64 + 64],
                        oph[:, :, 0:64],
                        rcp[:, q * 4:(q + 1) * 4, :].to_broadcast([128, 4, 64]),
                        op=mybir.AluOpType.mult)

            def proj_tr(b, p, osb, ott, on_pe=False):
                # O^T for hd-chunk p.  Mid-stream chunks ride the DMA XBAR
                # transpose unit (112ns DMA per 128x128 block, PE/DVE free);
                # the tail-critical last chunk uses PE transposes (the DMA
                # path's 8x(565 issue + 625 HWDGE) would stall the tail).
                if on_pe:
                    tps = psB.tile([128, T], bf16, tag="B", name=f"tps_{b}_{p}")
                    for tt in range(8):
                        nc.tensor.transpose(tps[:, tt * 128:tt * 128 + 128],
                                            osb[:, tt, p * 128:(p + 1) * 128], ident[:])
                    nc.vector.tensor_copy(ott[:, p, :], tps[:])
                    return
                for tt in range(8):
                    nc.sync.dma_start_transpose(ott[:, p, tt * 128:(tt + 1) * 128],
                                                osb[:, tt, p * 128:(p + 1) * 128])

            def proj_y(b, tt, ott, pool=None):
                yps = (pool or psB).tile([128, C], f32,
                                         tag="A" if pool is psA else "B",
                                         name=f"y_{b}_{tt}")
                for p in range(4):
                    nc.tensor.matmul(yps[:],
                                     ott[:, p, tt * 128:tt * 128 + 128],
                                     wp_s[:, p * 512:(p + 1) * 512],
                                     start=(p == 0), stop=(p == 3))
                ysb = y_pool.tile([128, C], bf16, tag="y", name=f"ys_{b}_{tt}")
                nc.vector.tensor_add(ysb[:], yps[:], bias_b[:])
                nc.sync.dma_start(y[b * T + tt * 128: b * T + tt * 128 + 128, :].bitcast(bf16),
                                  ysb[:])

            # ---------------- emission schedule ----------------
            # Priority rule (priority == emission order): the exp-feeding
            # chain scores(n+3) outranks PV(n), which outranks filler work.
            q0, k0, v0 = new_qkv_tiles(0)
            osb0 = o_pool.tile([128, 8, C], bf16, tag="o", name="osb_0")
            q1, k1, v1 = new_qkv_tiles(1)
            osb1 = o_pool.tile([128, 8, C], bf16, tag="o", name="osb_1")
            ott0 = ot_pool.tile([128, 4, T], bf16, tag="ot", name="ott_0")
            ott1 = ot_pool.tile([128, 4, T], bf16, tag="ot", name="ott_1")

            pts = {}
            qkv_qk_pair(0, 0, q0, k0, prologue=True)
            pts[0] = att_scores(0, 0, 0, q0, k0)
            pts[1] = att_scores(1, 0, 1, q0, k0)
            qkv_qk_pair(0, 1, q0, k0)
            pts[2] = att_scores(2, 0, 2, q0, k0)
            for j in range(8):
                qkv_v_tile(0, j, v0)

            def filler(n):
                # n = global head index 0..15; the non-critical work wave.
                if n < 2:
                    qkv_qk_pair(0, 2 + n, q0, k0)
                elif 3 <= n < 11:
                    qkv_qk_half(1, (n - 3) >> 1, (n - 3) & 1, q1, k1)
                if n < 8:
                    qkv_v_tile(1, n, v1)
                if n in (2, 4, 6, 8):
                    proj_tr(0, (n - 2) // 2, osb0, ott0)
                if 9 <= n < 15:
                    proj_y(0, n - 9, ott0)
                if n == 12:
                    proj_y(0, 6, ott0)
                if n == 13:
                    proj_y(0, 7, ott0)
                if n in (10, 12):
                    proj_tr(1, (n - 10) // 2, osb1, ott1)
                if n == 13:
                    proj_tr(1, 2, osb1, ott1)

            for n in range(16):
                b, h = n >> 3, n & 7
                if n < 13:
                    nb, nh = (n + 3) >> 3, (n + 3) & 7
                    pts[n + 3] = att_scores(n + 3, nb, nh, q0 if nb == 0 else q1,
                                            k0 if nb == 0 else k1)
                rcp = r_pool.tile([128, 8, 1], f32, tag="rc", name=f"rcp_{n}")
                att_pv(b, h, pts.pop(n), v0 if b == 0 else v1,
                       osb0 if b == 0 else osb1, rcp,
                       eager=(0, 1) if n == 15 else ())
                filler(n)

            # batch 1 projection tail; scores pool (psA) is idle by now
            proj_tr(1, 3, osb1, ott1, on_pe=True)
            for tt in range(8):
                proj_y(1, tt, ott1, pool=psB if tt % 2 else psA)

    nc.compile()
    return nc


def _pack_qk(w):
    # [NH, C, HD] -> [c, h*HD+d] -> tiled [c_local, p, cc, m] -> [128, 2048]
    wn = np.transpose(w, (1, 0, 2)).reshape(C, C)
    return np.ascontiguousarray(
        wn.reshape(4, 128, 4, 128).transpose(1, 2, 0, 3).reshape(128, 2048))


def _pack_cn(wn):
    # [C, N] natural -> tiled [c_local, cc, n] -> [128, 2048]
    return np.ascontiguousarray(wn.reshape(4, 128, C).transpose(1, 0, 2).reshape(128, 2048))


def get_nc():
    if "nc" not in _CACHE:
        _CACHE["nc"] = _build_nc()
    return _CACHE["nc"]


def make_in_maps(x, Wq, Wk, Wv, Wproj, bproj):
    x = np.asarray(x, dtype=np.float32)
    wqh, wql = _e4_split(_pack_qk(np.asarray(Wq, np.float32) * WSCALE))
    wkh, wkl = _e4_split(_pack_qk(np.asarray(Wk, np.float32) * WSCALE))
    wv_n = np.transpose(np.asarray(Wv, np.float32), (1, 0, 2)).reshape(C, C)
    wvh, wvl = _e4_split(_pack_cn(wv_n * WSCALE))
    wp_t = _bf16(_pack_cn(np.asarray(Wproj, np.float32) / WSCALE))
    bp_t = np.asarray(bproj, np.float32).reshape(1, C)
    in_maps = []
    for i in range(NCORES):
        xs = x[BL * i: BL * (i + 1)].reshape(BL, T, C)
        xt = np.ascontiguousarray(np.transpose(xs, (2, 0, 1)).reshape(C, BL * T))
        xth, xtl = _e4_split(xt)
        in_maps.append({
            "xh": xth, "xl": xtl,
            "wqh": wqh, "wql": wql, "wkh": wkh, "wkl": wkl,
            "wvh": wvh, "wvl": wvl, "wp": wp_t, "bp": bp_t,
        })
    return in_maps


def kernel(x, Wq, Wk, Wv, Wproj, bproj):
    from concourse.bass_utils import run_bass_kernel_spmd

    nc = get_nc()
    in_maps = make_in_maps(x, Wq, Wk, Wv, Wproj, bproj)
    trace = bool(int(os.environ.get("KERNEL_TRACE", "0")))
    res = run_bass_kernel_spmd(nc, in_maps, list(range(NCORES)), trace=trace)
    _CACHE["last_result"] = res
    out = np.empty((B, C, HH, WW), np.float32)
    for i in range(NCORES):
        yb = np.asarray(res.results[i]["y"]).view(np.uint16)
        yf = (yb.astype(np.uint32) << 16).view(np.float32)
        out[BL * i: BL * (i + 1)] = yf.reshape(BL, C, HH, WW)
    return out
